# revision 13
# baseline (speedup 1.0000x reference)
"""Trainium2 Bass kernel for nn_MixedLinearV2 (moe_routing).

y[b,s,o] = sum_i x[b,s,i] * (W[o,i]*coeff[o,i]) + b[o]*rowscale[o]

Strategy: data-parallel over batch (8 batch elements -> 8 NeuronCores).
W_mix = W*coeff and b_mix are precomputed on the HOST (cheap, depends
only on the 9 mixing weights + static masks), so the device kernel is a
pure GEMM + bias: per core y = x[c] @ W_mix^T + b_mix with
x [4096, 1024], W_mix [4096, 1024].

All matmul operands are bf16 (2048 matmuls of [128x128]x[128,512] per
core run at the 216 ns warm-issue floor; the 2-byte LDWEIGHTS stream
hides fully under the 512-column moving stream, unlike f32r).

DMA plan (the previous bottleneck): all tensors are laid out with >=
8 KB contiguous runs per partition so each descriptor moves 8 KB, and
the input stream (W chunks + x groups, in exact consumption order) is
issued on the Sync HWDGE ring while bias + y writebacks go on the
otherwise-idle Scalar (ACT) HWDGE ring. All of x (8 MB) and W (8 MB)
stay resident in SBUF. Output halves are evicted via DVE bias-add to
bf16 and written back per (s-tile, half); the host upcasts to f32.

Schedule: half 1 of each s-tile lags 4 s-tiles behind half 0, so W
chunks 4-7 are not needed until ~35 us into the run, hiding the W DMA
entirely behind compute.
"""

import sys
import types

import numpy as np
import ml_dtypes

# ---- constants (hardcoded from the problem spec) ----
B, S, IN, OUT = 8, 4096, 1024, 4096
IN_DIMS = (512, 768, 1024)
OUT_MULTS = (2, 3, 4)
P = 128
KT = IN // P          # 8 k-tiles
ST = S // P           # 32 s-tiles
OC = OUT // 512       # 8 out chunks of 512
G = 8                 # x groups of 4 s-tiles
GS = ST // G          # 4 s-tiles per group
LAG = 4               # half-1 s-tile lag
N_CORES = 8

MAIN_DT_NAME = "bf16"

BF16 = ml_dtypes.bfloat16


def _ensure_ntff_hook():
    """Register the antenv.axon_hooks shim so trace=True can profile."""
    if 'antenv.axon_hooks' in sys.modules:
        return
    try:
        import antenv
    except ImportError:
        return
    mod = types.ModuleType('antenv.axon_hooks')
    mod._hook = None
    mod.set_axon_ntff_profile_hook = lambda h: setattr(mod, '_hook', h)
    mod.get_axon_ntff_profile_hook = lambda: mod._hook
    sys.modules['antenv.axon_hooks'] = mod
    antenv.axon_hooks = mod
    try:
        from trn_agent_boot.trn_boot import _ntff_profile_via_ctypes
        mod.set_axon_ntff_profile_hook(
            _ntff_profile_via_ctypes('/opt/axon/libaxon_pjrt.so'))
    except Exception:
        pass


_BUILT = {}


def _build(main_dt_name=MAIN_DT_NAME):
    """Build + compile the SPMD Bass program (one program, 8 cores)."""
    if main_dt_name in _BUILT:
        return _BUILT[main_dt_name]
    assert main_dt_name == "bf16"

    import concourse.bacc as bacc
    import concourse.mybir as mybir
    from concourse.tile import TileContext

    F32 = mybir.dt.float32
    DT = mybir.dt.bfloat16

    nc = bacc.Bacc("TRN2", target_bir_lowering=False, debug=False,
                   num_devices=N_CORES)

    # xg[g, p, si, it, q] = x[(4g+si)*128+q, it*128+p] : 8KB/partition runs
    x_d = nc.declare_dram_parameter("xg", [G, P, GS, KT, P], DT,
                                    isOutput=False)
    # wT[p, oc, it, j] = W_mix[oc*512+j, it*128+p] : 8KB/partition per chunk
    wT_d = nc.declare_dram_parameter("WT", [P, OC, KT, 512], DT,
                                     isOutput=False)
    b_d = nc.declare_dram_parameter("b", [P, OUT], DT, isOutput=False)
    y_d = nc.declare_dram_parameter("y", [S, OUT], DT, isOutput=True)

    with TileContext(nc) as tc:
        with (
            tc.tile_pool(name="persist", bufs=1) as persist,
            tc.tile_pool(name="ysb_pool", bufs=6) as ysb_pool,
            tc.tile_pool(name="ps_pool", bufs=8, space="PSUM") as ps_pool,
        ):
            wmix = persist.tile([P, OC, KT, 512], DT)
            bias_sb = persist.tile([P, OUT], DT)
            xg_tiles = [persist.tile([P, GS, KT, P], DT, name=f"xg_{g}")
                        for g in range(G)]

            # Descriptor generation (~128 descs per full-partition DMA at
            # ~15-70ns each) is the DMA bottleneck, so: no sub-splits, every
            # DMA as wide as possible, and the head-critical transfers
            # spread across BOTH HWDGE rings so they generate in parallel:
            # Sync takes W even chunks + x groups, Scalar takes x group 0,
            # bias, W odd chunks, then all y writebacks.
            for ocx in range(OC):
                nc.sync.dma_start(wmix[:, ocx], wT_d[:, ocx])
            nc.sync.dma_start(xg_tiles[1][:], x_d[1])
            for g in range(2, G):
                nc.sync.dma_start(xg_tiles[g][:], x_d[g])

            nc.scalar.dma_start(xg_tiles[0][:], x_d[0])
            nc.scalar.dma_start(bias_sb[:, 0:2048], b_d[:, 0:2048])
            nc.scalar.dma_start(bias_sb[:, 2048:4096], b_d[:, 2048:4096])

            # Warm the PE HAM clock-gate (~3.4us of activity flips it from
            # 1.2 to 2.4 GHz) with throwaway matmuls on zeroed scratch while
            # the first input DMAs generate descriptors; results unread.
            # Enough of them (~10us worth) to bridge to data arrival, else
            # a >3.4us idle gap re-throttles before the real matmuls start.
            scratch = persist.tile([P, P], DT)
            nc.any.memzero(scratch[:])
            warm_ps = ps_pool.tile([P, 512], F32, tag="ps", name="warm")
            for _ in range(140):
                nc.tensor.matmul(warm_ps[:, 0:P], scratch[:], scratch[:],
                                 start=True, stop=True)

            def mm_chunk(s, ocx):
                """One psum chunk: 8 k-tile matmuls accumulating [P, 512]."""
                g, si = divmod(s, GS)
                yp = ps_pool.tile([P, 512], F32, tag="ps",
                                  name=f"yps_{s}_{ocx}")
                for it in range(KT):
                    nc.tensor.matmul(
                        yp[:], xg_tiles[g][:, si, it, :], wmix[:, ocx, it, :],
                        start=(it == 0), stop=(it == KT - 1))
                return yp

            def evict_chunk(s, ocx, yp, ysb):
                sl = slice(ocx * 512, (ocx + 1) * 512)
                nc.vector.tensor_tensor(ysb[:, sl], yp[:], bias_sb[:, sl],
                                        mybir.AluOpType.add)

            def unit(s, half, ysb):
                """k-tile-outer ordering: 4 consecutive matmuls share the
                same stationary x tile (identical LDWEIGHTS back to back,
                dedupable), accumulating into 4 psum banks at once."""
                g, si = divmod(s, GS)
                yps = [ps_pool.tile([P, 512], F32, tag="ps",
                                    name=f"yps_{s}_{half}_{j}")
                       for j in range(4)]
                for it in range(KT):
                    for j in range(4):
                        nc.tensor.matmul(
                            yps[j][:], xg_tiles[g][:, si, it, :],
                            wmix[:, half * 4 + j, it, :],
                            start=(it == 0), stop=(it == KT - 1),
                            skip_group_check=True)
                for j in range(4):
                    evict_chunk(s, half * 4 + j, yps[j], ysb)

            # Intro is chunk-major over s-tiles 0..3: one NEW W chunk per
            # ~7us of matmuls, matching the cold descriptor-generation rate
            # of the W stream, so the PE never waits on a chunk.
            ysb_rows = {s: ysb_pool.tile([P, OUT], DT, tag="ysb",
                                         name=f"ysb_{s}")
                        for s in range(LAG)}
            for j in range(4):
                for s in range(LAG):
                    evict_chunk(s, j, mm_chunk(s, j), ysb_rows[s])

            # Steady state: half 1 lags LAG s-tiles behind half 0; y rows
            # DMA out whole (128 x 8KB descriptors) once both halves
            # evicted. The last row's writeback is split per chunk so its
            # descriptor generation overlaps the final matmuls.
            for s in range(LAG, ST + LAG):
                if s < ST:
                    ysb_rows[s] = ysb_pool.tile([P, OUT], DT, tag="ysb",
                                                name=f"ysb_{s}")
                    unit(s, 0, ysb_rows[s])
                sp = s - LAG
                ysb = ysb_rows.pop(sp)
                unit(sp, 1, ysb)
                if sp < ST - 1:
                    nc.scalar.dma_start(y_d[sp * P:(sp + 1) * P, :], ysb[:])
                else:
                    # final row on the (idle by now) Sync ring: a single
                    # 128-descriptor generation right after the last evict
                    nc.sync.dma_start(y_d[sp * P:(sp + 1) * P, :], ysb[:])

    nc.compile()
    _BUILT[main_dt_name] = nc
    return nc


def _mix_np(weights, W, bias):
    """Host-side W_mix / b_mix (cheap: 4096x1024)."""
    out_dims = np.array([m * i for i in IN_DIMS for m in OUT_MULTS])
    in_dims = np.array([i for i in IN_DIMS for _ in OUT_MULTS])
    row_mask = (np.arange(OUT)[None, :] < out_dims[:, None]).astype(np.float32)
    col_mask = (np.arange(IN)[None, :] < in_dims[:, None]).astype(np.float32)
    cw = weights[:, None] * row_mask                    # [9, OUT]
    coeff = cw.T @ col_mask                             # [OUT, IN]
    W_mix = W * coeff
    b_mix = bias * (weights @ row_mask)
    return W_mix, b_mix


def _shard_layouts(inputs):
    """Host-side shard/layout prep: k-major bf16 tiles for x and W_mix."""
    x = np.asarray(inputs["x"], np.float32)
    weights = np.asarray(inputs["weights"], np.float32)
    W = np.asarray(inputs["W"], np.float32)
    bias = np.asarray(inputs["b"], np.float32)

    W_mix, b_mix = _mix_np(weights, W, bias)
    # wT[p, oc, it, j] = W_mix[oc*512+j, it*128+p]
    WT = np.ascontiguousarray(
        W_mix.reshape(OC, 512, KT, P).transpose(3, 0, 2, 1)).astype(BF16)
    b_bc = np.ascontiguousarray(
        np.broadcast_to(b_mix[None, :], (P, OUT))).astype(BF16)
    shared = {"WT": WT, "b": b_bc}
    in_maps = []
    for c in range(N_CORES):
        # xg[g, p, si, it, q] = x[c, ((g*4+si)*128)+q, it*128+p]
        xg = np.ascontiguousarray(
            x[c].reshape(G, GS, P, KT, P).transpose(0, 4, 1, 3, 2)
        ).astype(BF16)
        in_maps.append(dict(shared, xg=xg))
    return in_maps


def _run(inputs, main_dt_name=MAIN_DT_NAME, trace=False, tmpdir=None):
    _ensure_ntff_hook()
    import concourse.bass_utils as bass_utils
    # artifact upload needs a bucket; keep traces local
    bass_utils.upload_artifacts = lambda tmpdir: f"local:{tmpdir}"
    from concourse.bass_utils import run_bass_kernel_spmd

    nc = _build(main_dt_name)
    in_maps = _shard_layouts(inputs)
    res = run_bass_kernel_spmd(nc, in_maps, core_ids=list(range(N_CORES)),
                               trace=trace, tmpdir=tmpdir)
    y = np.empty((B, S, OUT), np.float32)
    for c in range(N_CORES):
        y[c] = res.results[c]["y"].astype(np.float32)
    return y, res


def kernel(**inputs) -> np.ndarray:
    y, _ = _run(inputs, trace=False)
    return y


# revision 14
# speedup vs baseline: 1.0055x; 1.0055x over previous
"""Trainium2 Bass kernel for nn_MixedLinearV2 (moe_routing).

y[b,s,o] = sum_i x[b,s,i] * (W[o,i]*coeff[o,i]) + b[o]*rowscale[o]

Strategy: data-parallel over batch (8 batch elements -> 8 NeuronCores).
W_mix = W*coeff and b_mix are precomputed on the HOST (cheap, depends
only on the 9 mixing weights + static masks), so the device kernel is a
pure GEMM + bias: per core y = x[c] @ W_mix^T + b_mix with
x [4096, 1024], W_mix [4096, 1024].

All matmul operands are bf16 (2048 matmuls of [128x128]x[128,512] per
core run at the 216 ns warm-issue floor; the 2-byte LDWEIGHTS stream
hides fully under the 512-column moving stream, unlike f32r).

DMA plan (the previous bottleneck): all tensors are laid out with >=
8 KB contiguous runs per partition so each descriptor moves 8 KB, and
the input stream (W chunks + x groups, in exact consumption order) is
issued on the Sync HWDGE ring while bias + y writebacks go on the
otherwise-idle Scalar (ACT) HWDGE ring. All of x (8 MB) and W (8 MB)
stay resident in SBUF. Output halves are evicted via DVE bias-add to
bf16 and written back per (s-tile, half); the host upcasts to f32.

Schedule: half 1 of each s-tile lags 4 s-tiles behind half 0, so W
chunks 4-7 are not needed until ~35 us into the run, hiding the W DMA
entirely behind compute.
"""

import sys
import types

import numpy as np
import ml_dtypes

# ---- constants (hardcoded from the problem spec) ----
B, S, IN, OUT = 8, 4096, 1024, 4096
IN_DIMS = (512, 768, 1024)
OUT_MULTS = (2, 3, 4)
P = 128
KT = IN // P          # 8 k-tiles
ST = S // P           # 32 s-tiles
OC = OUT // 512       # 8 out chunks of 512
G = 8                 # x groups of 4 s-tiles
GS = ST // G          # 4 s-tiles per group
LAG = 4               # half-1 s-tile lag
N_CORES = 8

MAIN_DT_NAME = "bf16"

BF16 = ml_dtypes.bfloat16


def _ensure_ntff_hook():
    """Register the antenv.axon_hooks shim so trace=True can profile."""
    if 'antenv.axon_hooks' in sys.modules:
        return
    try:
        import antenv
    except ImportError:
        return
    mod = types.ModuleType('antenv.axon_hooks')
    mod._hook = None
    mod.set_axon_ntff_profile_hook = lambda h: setattr(mod, '_hook', h)
    mod.get_axon_ntff_profile_hook = lambda: mod._hook
    sys.modules['antenv.axon_hooks'] = mod
    antenv.axon_hooks = mod
    try:
        from trn_agent_boot.trn_boot import _ntff_profile_via_ctypes
        mod.set_axon_ntff_profile_hook(
            _ntff_profile_via_ctypes('/opt/axon/libaxon_pjrt.so'))
    except Exception:
        pass


_BUILT = {}


def _build(main_dt_name=MAIN_DT_NAME):
    """Build + compile the SPMD Bass program (one program, 8 cores)."""
    if main_dt_name in _BUILT:
        return _BUILT[main_dt_name]
    assert main_dt_name == "bf16"

    import concourse.bacc as bacc
    import concourse.mybir as mybir
    from concourse.tile import TileContext

    F32 = mybir.dt.float32
    DT = mybir.dt.bfloat16

    nc = bacc.Bacc("TRN2", target_bir_lowering=False, debug=False,
                   num_devices=N_CORES)

    # xg[g, p, si, it, q] = x[(4g+si)*128+q, it*128+p] : 8KB/partition runs
    x_d = nc.declare_dram_parameter("xg", [G, P, GS, KT, P], DT,
                                    isOutput=False)
    # wT[p, oc, it, j] = W_mix[oc*512+j, it*128+p] : 8KB/partition per chunk
    wT_d = nc.declare_dram_parameter("WT", [P, OC, KT, 512], DT,
                                     isOutput=False)
    b_d = nc.declare_dram_parameter("b", [P, OUT], DT, isOutput=False)
    y_d = nc.declare_dram_parameter("y", [S, OUT], DT, isOutput=True)

    with TileContext(nc) as tc:
        with (
            tc.tile_pool(name="persist", bufs=1) as persist,
            tc.tile_pool(name="ysb_pool", bufs=6) as ysb_pool,
            tc.tile_pool(name="ps_pool", bufs=8, space="PSUM") as ps_pool,
        ):
            wmix = persist.tile([P, OC, KT, 512], DT)
            bias_sb = persist.tile([P, OUT], DT)
            xg_tiles = [persist.tile([P, GS, KT, P], DT, name=f"xg_{g}")
                        for g in range(G)]

            # Descriptor generation (~128 descs per full-partition DMA at
            # ~15-70ns each) is the DMA bottleneck, so: no sub-splits, every
            # DMA as wide as possible, and the head-critical transfers
            # spread across BOTH HWDGE rings so they generate in parallel:
            # Sync takes W even chunks + x groups, Scalar takes x group 0,
            # bias, W odd chunks, then all y writebacks.
            for ocx in range(OC):
                nc.sync.dma_start(wmix[:, ocx], wT_d[:, ocx])
            nc.sync.dma_start(xg_tiles[1][:], x_d[1])
            for g in range(2, G):
                nc.sync.dma_start(xg_tiles[g][:], x_d[g])

            nc.scalar.dma_start(xg_tiles[0][:], x_d[0])
            nc.scalar.dma_start(bias_sb[:, 0:2048], b_d[:, 0:2048])
            nc.scalar.dma_start(bias_sb[:, 2048:4096], b_d[:, 2048:4096])

            # Warm the PE HAM clock-gate (~3.4us of activity flips it from
            # 1.2 to 2.4 GHz) with throwaway matmuls on zeroed scratch while
            # the first input DMAs generate descriptors; results unread.
            # Enough of them (~10us worth) to bridge to data arrival, else
            # a >3.4us idle gap re-throttles before the real matmuls start.
            scratch = persist.tile([P, P], DT)
            nc.any.memzero(scratch[:])
            warm_ps = ps_pool.tile([P, 512], F32, tag="ps", name="warm")
            for _ in range(140):
                nc.tensor.matmul(warm_ps[:, 0:P], scratch[:], scratch[:],
                                 start=True, stop=True)

            def mm_chunk(s, ocx):
                """One psum chunk: 8 k-tile matmuls accumulating [P, 512]."""
                g, si = divmod(s, GS)
                yp = ps_pool.tile([P, 512], F32, tag="ps",
                                  name=f"yps_{s}_{ocx}")
                for it in range(KT):
                    nc.tensor.matmul(
                        yp[:], xg_tiles[g][:, si, it, :], wmix[:, ocx, it, :],
                        start=(it == 0), stop=(it == KT - 1))
                return yp

            def evict_chunk(s, ocx, yp, ysb):
                sl = slice(ocx * 512, (ocx + 1) * 512)
                nc.vector.tensor_tensor(ysb[:, sl], yp[:], bias_sb[:, sl],
                                        mybir.AluOpType.add)

            def unit(s, half, ysb):
                for j in range(4):
                    ocx = half * 4 + j
                    evict_chunk(s, ocx, mm_chunk(s, ocx), ysb)

            # Intro is chunk-major over s-tiles 0..3: one NEW W chunk per
            # ~7us of matmuls, matching the cold descriptor-generation rate
            # of the W stream, so the PE never waits on a chunk.
            ysb_rows = {s: ysb_pool.tile([P, OUT], DT, tag="ysb",
                                         name=f"ysb_{s}")
                        for s in range(LAG)}
            for j in range(4):
                for s in range(LAG):
                    evict_chunk(s, j, mm_chunk(s, j), ysb_rows[s])

            # Steady state: half 1 lags LAG s-tiles behind half 0; y rows
            # DMA out whole (128 x 8KB descriptors) once both halves
            # evicted. The last row's writeback is split per chunk so its
            # descriptor generation overlaps the final matmuls.
            for s in range(LAG, ST + LAG):
                if s < ST:
                    ysb_rows[s] = ysb_pool.tile([P, OUT], DT, tag="ysb",
                                                name=f"ysb_{s}")
                    unit(s, 0, ysb_rows[s])
                sp = s - LAG
                ysb = ysb_rows.pop(sp)
                unit(sp, 1, ysb)
                if sp < ST - 1:
                    nc.scalar.dma_start(y_d[sp * P:(sp + 1) * P, :], ysb[:])
                else:
                    # final row on the (idle by now) Sync ring: a single
                    # 128-descriptor generation right after the last evict
                    nc.sync.dma_start(y_d[sp * P:(sp + 1) * P, :], ysb[:])

    nc.compile()
    _BUILT[main_dt_name] = nc
    return nc


def _mix_np(weights, W, bias):
    """Host-side W_mix / b_mix (cheap: 4096x1024)."""
    out_dims = np.array([m * i for i in IN_DIMS for m in OUT_MULTS])
    in_dims = np.array([i for i in IN_DIMS for _ in OUT_MULTS])
    row_mask = (np.arange(OUT)[None, :] < out_dims[:, None]).astype(np.float32)
    col_mask = (np.arange(IN)[None, :] < in_dims[:, None]).astype(np.float32)
    cw = weights[:, None] * row_mask                    # [9, OUT]
    coeff = cw.T @ col_mask                             # [OUT, IN]
    W_mix = W * coeff
    b_mix = bias * (weights @ row_mask)
    return W_mix, b_mix


def _shard_layouts(inputs):
    """Host-side shard/layout prep: k-major bf16 tiles for x and W_mix."""
    x = np.asarray(inputs["x"], np.float32)
    weights = np.asarray(inputs["weights"], np.float32)
    W = np.asarray(inputs["W"], np.float32)
    bias = np.asarray(inputs["b"], np.float32)

    W_mix, b_mix = _mix_np(weights, W, bias)
    # wT[p, oc, it, j] = W_mix[oc*512+j, it*128+p]
    WT = np.ascontiguousarray(
        W_mix.reshape(OC, 512, KT, P).transpose(3, 0, 2, 1)).astype(BF16)
    b_bc = np.ascontiguousarray(
        np.broadcast_to(b_mix[None, :], (P, OUT))).astype(BF16)
    shared = {"WT": WT, "b": b_bc}
    in_maps = []
    for c in range(N_CORES):
        # xg[g, p, si, it, q] = x[c, ((g*4+si)*128)+q, it*128+p]
        xg = np.ascontiguousarray(
            x[c].reshape(G, GS, P, KT, P).transpose(0, 4, 1, 3, 2)
        ).astype(BF16)
        in_maps.append(dict(shared, xg=xg))
    return in_maps


def _run(inputs, main_dt_name=MAIN_DT_NAME, trace=False, tmpdir=None):
    _ensure_ntff_hook()
    import concourse.bass_utils as bass_utils
    # artifact upload needs a bucket; keep traces local
    bass_utils.upload_artifacts = lambda tmpdir: f"local:{tmpdir}"
    from concourse.bass_utils import run_bass_kernel_spmd

    nc = _build(main_dt_name)
    in_maps = _shard_layouts(inputs)
    res = run_bass_kernel_spmd(nc, in_maps, core_ids=list(range(N_CORES)),
                               trace=trace, tmpdir=tmpdir)
    y = np.empty((B, S, OUT), np.float32)
    for c in range(N_CORES):
        y[c] = res.results[c]["y"].astype(np.float32)
    return y, res


def kernel(**inputs) -> np.ndarray:
    y, _ = _run(inputs, trace=False)
    return y
